# revision 20
# baseline (speedup 1.0000x reference)
"""Trainium2 Bass kernel for the BEMv13 MoE-LoRA module (bf16 edition).

Computation (per token t, full problem):
  base  = x @ W_base.T + b_base
  w     = softmax(x @ W_router + b_router)        # E=2 experts
  H     = x @ A_cat.T                             # [T, 16] LoRA down-proj, both experts
  G     = H * w_broadcast * (alpha/rank)          # per-expert routing weight
  out   = base + G @ B_cat.T

Sharding: tokens (batch*seq = 16384) split evenly across 8 NeuronCores;
all weights replicated. No cross-core communication.

On-core algorithm (per core, 2048 tokens):
  - All matmul operands are bf16 (1 PE cycle/row, same as fp32r, but half
    the DMA traffic and no on-chip casts); rel err ~2e-3 vs the 2e-2 gate.
  - x is pre-transposed AND pre-tiled on the host: dram row block t holds
    the 16 stationary lhsT tiles [k=128, tok=128] for token-tile t,
    contiguous, so one 4KB/line DMA per tile feeds the PE directly.
    This removes all 256 PE transposes of the fp32r design.
  - W^T is pre-packed per k-slab [128, KT*O] bf16 (64KB/partition),
    resident in SBUF, streamed as 32 half-slabs on the SP queue.
  - Startup: tiles 0 and 1 are processed as a PAIR over half of O at a
    time (4 PSUM acc banks), zippered k-by-k in W DMA arrival order, so
    PE consumption (~0.87us/half-slab) tracks the DMA stream
    (~0.79us/half-slab) and the PE never idles waiting for W.
  - The LoRA/router accumulator h START includes k-slabs 14,15 in a
    pre-block at tile start and STOPS at k=13, so the routing-weight
    chain (2 ACT sigmoids -> 2 DVE muls -> PE transpose -> DVE copy)
    overlaps the last two main matmul groups instead of stalling the PE.
  - softmax over 2 experts == sigmoid of the logit difference; both
    w0=sigmoid(-d) and w1=sigmoid(d) are produced directly on ACT
    (LORA_SCALE is folded into B_cat^T on the host).
  - G^T is a final K=16 accumulation step into the same PSUM banks.
  - b_base is added on the HOST (free); drains are pure PSUM->SBUF
    copies to bf16, split 2 on DVE + 2 on ACT, so the PSUM bank
    rotation never throttles the next tile's matmuls. Output DMAs
    (bf16, host upconverts) ride the Pool queue in column halves.
"""

import numpy as np

P = 128
D = 2048
O = 2048
KT = D // P            # 16 k-tiles
TOK = 2048             # tokens per core
NT = TOK // P          # 16 token tiles
HN = 18                # 16 LoRA cols + 1 router-diff col + 1 pad
ER = 16                # E*R
HO = O // 2            # half of O for the startup pair phase
SCALE = 16.0 / 8.0
NCORES = 8

_CACHE = {}


def _build():
    import concourse.tile as tile
    import concourse.masks as masks
    from concourse import bacc, mybir

    f32 = mybir.dt.float32
    bf16 = mybir.dt.bfloat16

    nc = bacc.Bacc("TRN2", target_bir_lowering=False, debug=False)

    # xt: row block t = the 16 stationary lhsT tiles of token-tile t,
    # xt[t*P + p, k*P + j] = x[t*P + j, k*P + p]
    xt_d = nc.dram_tensor("xt", [TOK, D], bf16, kind="ExternalInput")
    # wt: wt[p, k*O + o] = W_base[o, k*P + p]
    wt_d = nc.dram_tensor("wt", [P, KT * O], bf16, kind="ExternalInput")
    aat_d = nc.dram_tensor("aat", [P, KT * HN], bf16, kind="ExternalInput")
    bt_d = nc.dram_tensor("bt", [ER, O], bf16, kind="ExternalInput")
    brd_d = nc.dram_tensor("brd", [1, 2], f32, kind="ExternalInput")
    out_d = nc.dram_tensor("out", [TOK, O], bf16, kind="ExternalOutput")

    with tile.TileContext(nc) as tc:
        with (
            tc.tile_pool(name="res", bufs=1) as res,
            tc.tile_pool(name="obuf", bufs=2) as obuf,
            tc.tile_pool(name="xpool", bufs=4) as xpool,
            tc.tile_pool(name="small", bufs=2) as small,
            tc.tile_pool(name="psA", bufs=5, space="PSUM") as psA,
            tc.tile_pool(name="psT", bufs=1, space="PSUM") as psT,
            tc.tile_pool(name="psH", bufs=2, space="PSUM") as psH,
        ):
            ident = res.tile([P, P], f32, tag="ident")
            masks.make_identity(nc, ident[:])

            # --- W^T stream: SP queue, half-slabs in (half, k) order so the
            # startup pair phase consumes them in arrival order. (Keeping
            # the SP queue W-only matters: routing aat/x0/x1 through it
            # delayed the first PE op by ~5us — waits on that queue appear
            # to resolve against later completions.)
            wt_b = res.tile([P, KT * O], bf16, tag="wt_b")
            x_tiles = [None] * NT

            def load_x(t, chunks=1, queue=None):
                x_tiles[t] = xpool.tile([P, D], bf16, tag="x", name=f"x_{t}")
                cw = D // chunks
                for cc in range(chunks):
                    (queue or nc.scalar).dma_start(
                        x_tiles[t][:, cc * cw:(cc + 1) * cw],
                        xt_d[t * P:(t + 1) * P, cc * cw:(cc + 1) * cw])

            def wdma(lo, hi):
                nc.sync.dma_start(wt_b[:, lo:hi], wt_d[:, lo:hi])

            # W-A halves; the very first in quarters so the first matmul's
            # W arrives ~0.4us earlier
            wdma(0, 512)
            wdma(512, HO)
            for k in range(1, KT):
                wdma(k * O, k * O + HO)
            # x2/x3 ride the SP queue between the A and B halves: their
            # transfers land after phase A's W stream instead of competing
            # with it for HBM, and well before tiles 2/3 need them.
            load_x(2, queue=nc.sync)
            load_x(3, queue=nc.sync)
            for k in range(KT):
                wdma(k * O + HO, (k + 1) * O)

            # x0/x1 + small constants on the ACT HWDGE queue; natural chunk
            # order: the scheduler starts the pair phase's k=0 acc matmuls
            # on (x0 chunk 0 + W quarter 0) and slots the h pre-block in
            # once aat + chunk 3 land.
            load_x(0, chunks=4)
            load_x(1, chunks=2)

            aat_b = res.tile([P, KT * HN], bf16, tag="aat_b")
            nc.scalar.dma_start(aat_b[:], aat_d[:])
            bt_b = res.tile([ER, O], bf16, tag="bt_b")
            nc.scalar.dma_start(bt_b[:], bt_d[:])
            # router bias diff [+d, -d], partition-broadcast
            brd128 = res.tile([P, 2], f32, tag="brd128")
            nc.gpsimd.dma_start(brd128[:], brd_d[:].broadcast_to((P, 2)))

            def lhs(t, k):
                return x_tiles[t][:, k * P:(k + 1) * P]

            # routing weights + scaled-H transpose; returns gt [ER, P] bf16
            def make_gt(t, h):
                w1s = small.tile([P, 1], f32, tag="w1s", name=f"w1s_{t}")
                nc.scalar.activation(w1s[:], h[:, ER:ER + 1],
                                     mybir.ActivationFunctionType.Sigmoid,
                                     bias=brd128[:, 0:1], scale=1.0)
                w0s = small.tile([P, 1], f32, tag="w0s", name=f"w0s_{t}")
                nc.scalar.activation(w0s[:], h[:, ER:ER + 1],
                                     mybir.ActivationFunctionType.Sigmoid,
                                     bias=brd128[:, 1:2], scale=-1.0)
                g = small.tile([P, ER], f32, tag="g", name=f"g_{t}")
                nc.vector.tensor_scalar_mul(g[:, 0:8], h[:, 0:8], w0s[:])
                nc.vector.tensor_scalar_mul(g[:, 8:16], h[:, 8:16], w1s[:])
                gst = psT.tile([ER, P], f32, tag="gst", name=f"gst_{t}")
                nc.tensor.transpose(gst[:], g[:], ident[:])
                gt = small.tile([ER, P], bf16, tag="gt", name=f"gt_{t}")
                nc.vector.tensor_copy(gt[:], gst[:])
                return gt

            # h pre-block: open the h accumulation with k-slabs 14,15 so the
            # in-loop h stops at k=13, two matmul groups before the loop ends.
            def h_preblock(t, h):
                nc.tensor.matmul(h[:], lhs(t, KT - 2),
                                 aat_b[:, (KT - 2) * HN:(KT - 1) * HN],
                                 start=True, stop=False)
                nc.tensor.matmul(h[:], lhs(t, KT - 1),
                                 aat_b[:, (KT - 1) * HN:KT * HN],
                                 start=False, stop=False)

            # drain acc j of tile t as a pure copy (bias added on host);
            # even j on DVE, odd j on ACT, so the two engines split the work.
            def drain(t, acc, j):
                outt = out_tiles[t]
                dst = outt[:, j * 512:(j + 1) * 512]
                if j % 2 == 0:
                    nc.vector.tensor_copy(dst, acc[:])
                else:
                    nc.scalar.copy(dst, acc[:])

            def store(t, half):
                nc.gpsimd.dma_start(
                    out_d[t * P:(t + 1) * P, half * HO:(half + 1) * HO],
                    out_tiles[t][:, half * HO:(half + 1) * HO])

            out_tiles = [None] * NT

            # =========== startup: tiles 0,1 as a pair, half-O per pass =====
            hps = [psH.tile([P, HN], f32, tag="h", name=f"h_{t}")
                   for t in range(2)]
            gts = [None, None]
            for t in range(2):
                out_tiles[t] = obuf.tile([P, O], bf16, tag="obuf", name=f"out_{t}")

            for hh in range(2):
                accs = [[psA.tile([P, 512], f32, tag="acc", name=f"acc_{t}_{hh}_{j}")
                         for j in range(2)] for t in range(2)]
                if hh == 0:
                    for t in range(2):
                        h_preblock(t, hps[t])
                for k in range(KT):
                    for t in range(2):
                        if hh == 0 and k < KT - 2:
                            nc.tensor.matmul(hps[t][:], lhs(t, k),
                                             aat_b[:, k * HN:(k + 1) * HN],
                                             start=False, stop=(k == KT - 3))
                        for j in range(2):
                            nc.tensor.matmul(
                                accs[t][j][:], lhs(t, k),
                                wt_b[:, k * O + hh * HO + j * 512:
                                     k * O + hh * HO + (j + 1) * 512],
                                start=(k == 0), stop=False)
                    if hh == 0 and k == KT - 2:
                        # both h's stopped at k=13 (pre-block covers 14,15):
                        # both chains overlap the k=14,15 matmul groups
                        gts[0] = make_gt(0, hps[0])
                        gts[1] = make_gt(1, hps[1])
                for t in range(2):
                    for j in range(2):
                        nc.tensor.matmul(accs[t][j][:], gts[t][:],
                                         bt_b[:, hh * HO + j * 512:
                                              hh * HO + (j + 1) * 512],
                                         start=False, stop=True)
                for t in range(2):
                    for j in range(2):
                        drain(t, accs[t][j], 2 * hh + j)
                for t in range(2):
                    store(t, hh)

            # =========== main loop: tiles 2..15, one tile at a time ========
            for t in range(2, NT):
                if t + 2 < NT:
                    load_x(t + 2)
                out_tiles[t] = obuf.tile([P, O], bf16, tag="obuf", name=f"out_{t}")
                accs = [psA.tile([P, 512], f32, tag="acc", name=f"acc_{t}_{j}")
                        for j in range(4)]
                h = psH.tile([P, HN], f32, tag="h", name=f"h_{t}")
                h_preblock(t, h)
                gt = None
                for k in range(KT):
                    if k < KT - 2:
                        nc.tensor.matmul(h[:], lhs(t, k),
                                         aat_b[:, k * HN:(k + 1) * HN],
                                         start=False, stop=(k == KT - 3))
                    for j in range(4):
                        nc.tensor.matmul(
                            accs[j][:], lhs(t, k),
                            wt_b[:, k * O + j * 512:k * O + (j + 1) * 512],
                            start=(k == 0), stop=False)
                    if k == KT - 2:
                        gt = make_gt(t, h)
                for j in range(4):
                    nc.tensor.matmul(accs[j][:], gt[:],
                                     bt_b[:, j * 512:(j + 1) * 512],
                                     start=False, stop=True)
                # drain order 0(DVE),1(ACT),2(DVE),3(ACT); store each half
                # as soon as both of its column chunks are in SBUF. The
                # last tile stores per-quarter to shorten the drain tail.
                if t == NT - 1:
                    # quarter stores on two idle queues -> parallel transfers
                    for j, q in zip(range(4),
                                    (nc.gpsimd, nc.sync, nc.gpsimd, nc.sync)):
                        drain(t, accs[j], j)
                        q.dma_start(
                            out_d[t * P:(t + 1) * P, j * 512:(j + 1) * 512],
                            out_tiles[t][:, j * 512:(j + 1) * 512])
                else:
                    drain(t, accs[0], 0)
                    drain(t, accs[1], 1)
                    store(t, 0)
                    drain(t, accs[2], 2)
                    drain(t, accs[3], 3)
                    store(t, 1)

    nc.compile()
    return nc


def _prep_host(x, W_base, b_base, A, B, W_router, b_router):
    """Host-side layout prep + sharding. Returns per-core input maps."""
    import ml_dtypes
    bf16 = ml_dtypes.bfloat16

    x_flat = np.ascontiguousarray(x, dtype=np.float32).reshape(-1, D)
    # xt[t*P + p, k*P + j] = x[t*P + j, k*P + p], per core
    NTOT = x_flat.shape[0] // P
    xt_all = np.ascontiguousarray(
        x_flat.reshape(NTOT, P, KT, P).transpose(0, 3, 2, 1)
    ).reshape(NTOT * P, D).astype(bf16)

    wt = np.asarray(W_base, dtype=np.float32).T                      # [D, O]
    wt_p = np.ascontiguousarray(
        wt.reshape(KT, P, O).transpose(1, 0, 2).reshape(P, KT * O)
    ).astype(bf16)

    a_cat = np.asarray(A, dtype=np.float32).reshape(ER, D)           # [16, D]
    aat = np.zeros((D, HN), dtype=np.float32)
    aat[:, :ER] = a_cat.T
    wr = np.asarray(W_router, dtype=np.float32)
    aat[:, ER] = wr[:, 1] - wr[:, 0]
    aat_p = np.ascontiguousarray(
        aat.reshape(KT, P, HN).transpose(1, 0, 2).reshape(P, KT * HN)
    ).astype(bf16)

    b_cat = np.concatenate([np.asarray(B, dtype=np.float32)[0],
                            np.asarray(B, dtype=np.float32)[1]], axis=1)  # [O, 16]
    bt = np.ascontiguousarray(b_cat.T * SCALE).astype(bf16)          # [16, O]
    dlb = np.float32(b_router[1]) - np.float32(b_router[0])
    brd = np.array([[dlb, -dlb]], dtype=np.float32)

    in_maps = []
    for c in range(NCORES):
        in_maps.append({
            "xt": xt_all[c * TOK:(c + 1) * TOK],
            "wt": wt_p,
            "aat": aat_p,
            "bt": bt,
            "brd": brd,
        })
    return in_maps


def kernel(x, W_base, b_base, A, B, W_router, b_router):
    from concourse import bass_utils

    # NOTE: walrus --enable-ldw-opt=true rejects bf16 LDWEIGHTS
    # ("InstLdweights is not compatible with LDW optimization"); measured
    # on HW the un-deduped loads fully hide under the matmul streams.
    if "nc" not in _CACHE:
        _CACHE["nc"] = _build()
    nc = _CACHE["nc"]

    in_maps = _prep_host(x, W_base, b_base, A, B, W_router, b_router)
    res = None
    for attempt in range(3):
        try:
            res = bass_utils.run_bass_kernel_spmd(
                nc, in_maps, core_ids=list(range(NCORES)))
            break
        except Exception:
            # rare transient NRT_EXEC_UNIT_UNRECOVERABLE observed once;
            # the same NEFF runs fine on retry
            if attempt == 2:
                raise
    out = np.concatenate([res.results[c]["out"] for c in range(NCORES)], axis=0)
    out = out.astype(np.float32) + np.asarray(b_base, dtype=np.float32)
    return out.reshape(np.asarray(x).shape[0], -1, O)


# revision 32
# speedup vs baseline: 1.2394x; 1.2394x over previous
"""Trainium2 Bass kernel for the BEMv13 MoE-LoRA module (bf16 edition).

Computation (per token t, full problem):
  base  = x @ W_base.T + b_base
  w     = softmax(x @ W_router + b_router)        # E=2 experts
  H     = x @ A_cat.T                             # [T, 16] LoRA down-proj, both experts
  G     = H * w_broadcast * (alpha/rank)          # per-expert routing weight
  out   = base + G @ B_cat.T

Sharding: tokens (batch*seq = 16384) split evenly across 8 NeuronCores;
all weights replicated. No cross-core communication.

On-core algorithm (per core, 2048 tokens):
  - All matmul operands are bf16 (1 PE cycle/row, same as fp32r, but half
    the DMA traffic and no on-chip casts); rel err ~2e-3 vs the 2e-2 gate.
  - x is pre-transposed AND pre-tiled on the host: dram row block t holds
    the 16 stationary lhsT tiles [k=128, tok=128] for token-tile t,
    contiguous, so one 4KB/line DMA per tile feeds the PE directly.
    This removes all 256 PE transposes of the fp32r design.
  - W^T is pre-packed per k-slab [128, KT*O] bf16 (64KB/partition),
    resident in SBUF, streamed as 32 half-slabs on the SP queue.
  - Startup: tiles 0 and 1 are processed as a PAIR over half of O at a
    time (4 PSUM acc banks), zippered k-by-k in W DMA arrival order, so
    PE consumption (~0.87us/half-slab) tracks the DMA stream
    (~0.79us/half-slab) and the PE never idles waiting for W.
  - The LoRA/router accumulator h START includes k-slabs 14,15 in a
    pre-block at tile start and STOPS at k=13, so the routing-weight
    chain (2 ACT sigmoids -> 2 DVE muls -> PE transpose -> DVE copy)
    overlaps the last two main matmul groups instead of stalling the PE.
  - softmax over 2 experts == sigmoid of the logit difference; both
    w0=sigmoid(-d) and w1=sigmoid(d) are produced directly on ACT
    (LORA_SCALE is folded into B_cat^T on the host).
  - G^T is a final K=16 accumulation step into the same PSUM banks.
  - b_base is added on the HOST (free); drains are pure PSUM->SBUF
    copies to bf16, split 2 on DVE + 2 on ACT, so the PSUM bank
    rotation never throttles the next tile's matmuls. Output DMAs
    (bf16, host upconverts) ride the Pool queue in column halves.
"""

import numpy as np

P = 128
D = 2048
O = 2048
KT = D // P            # 16 k-tiles
TOK = 2048             # tokens per core
NT = TOK // P          # 16 token tiles
HN = 18                # 16 LoRA cols + 1 router-diff col + 1 pad
ER = 16                # E*R
HO = O // 2            # half of O for the startup pair phase
SCALE = 16.0 / 8.0
NCORES = 8

_CACHE = {}


def _build():
    import concourse.tile as tile
    import concourse.masks as masks
    from concourse import bacc, mybir

    f32 = mybir.dt.float32
    bf16 = mybir.dt.bfloat16

    nc = bacc.Bacc("TRN2", target_bir_lowering=False, debug=False)

    # xt: row block t = the 16 stationary lhsT tiles of token-tile t,
    # xt[t*P + p, k*P + j] = x[t*P + j, k*P + p]
    xt_d = nc.dram_tensor("xt", [TOK, D], bf16, kind="ExternalInput")
    # wt: wt[p, k*O + o] = W_base[o, k*P + p]
    wt_d = nc.dram_tensor("wt", [P, KT * O], bf16, kind="ExternalInput")
    aat_d = nc.dram_tensor("aat", [P, KT * HN], bf16, kind="ExternalInput")
    bt_d = nc.dram_tensor("bt", [ER, O], bf16, kind="ExternalInput")
    brd_d = nc.dram_tensor("brd", [1, 2], f32, kind="ExternalInput")
    out_d = nc.dram_tensor("out", [TOK, O], bf16, kind="ExternalOutput")

    with tile.TileContext(nc) as tc:
        with (
            tc.tile_pool(name="res", bufs=1) as res,
            tc.tile_pool(name="obuf", bufs=2) as obuf,
            tc.tile_pool(name="xpool", bufs=3) as xpool,
            tc.tile_pool(name="small", bufs=2) as small,
            tc.tile_pool(name="psA", bufs=5, space="PSUM") as psA,
            tc.tile_pool(name="psT", bufs=1, space="PSUM") as psT,
            tc.tile_pool(name="psH", bufs=2, space="PSUM") as psH,
        ):
            ident = res.tile([P, P], f32, tag="ident")
            masks.make_identity(nc, ident[:])

            # --- W^T stream: SP queue, half-slabs in (half, k) order so the
            # startup pair phase consumes them in arrival order. (Keeping
            # the SP queue W-only matters: routing aat/x0/x1 through it
            # delayed the first PE op by ~5us — waits on that queue appear
            # to resolve against later completions.)
            wt_b = res.tile([P, KT * O], bf16, tag="wt_b")
            for hh in range(2):
                for k in range(KT):
                    nc.sync.dma_start(
                        wt_b[:, k * O + hh * HO:k * O + (hh + 1) * HO],
                        wt_d[:, k * O + hh * HO:k * O + (hh + 1) * HO])

            # x tok-tile loads + small constants on the ACT HWDGE queue.
            x_tiles = [None] * NT

            def load_x(t, chunks=1, order=None):
                x_tiles[t] = xpool.tile([P, D], bf16, tag="x", name=f"x_{t}")
                cw = D // chunks
                for cc in (order or range(chunks)):
                    nc.scalar.dma_start(
                        x_tiles[t][:, cc * cw:(cc + 1) * cw],
                        xt_d[t * P:(t + 1) * P, cc * cw:(cc + 1) * cw])

            # natural chunk order: the scheduler starts the pair phase's k=0
            # acc matmuls on (x0 chunk 0 + W slab 0) and slots the h
            # pre-block in once aat + chunk 3 land.
            load_x(0, chunks=4)
            load_x(1, chunks=2)

            aat_b = res.tile([P, KT * HN], bf16, tag="aat_b")
            nc.scalar.dma_start(aat_b[:], aat_d[:])
            bt_b = res.tile([ER, O], bf16, tag="bt_b")
            nc.scalar.dma_start(bt_b[:], bt_d[:])
            # router bias diff [+d, -d], partition-broadcast
            brd128 = res.tile([P, 2], f32, tag="brd128")
            nc.gpsimd.dma_start(brd128[:], brd_d[:].broadcast_to((P, 2)))

            def lhs(t, k):
                return x_tiles[t][:, k * P:(k + 1) * P]

            # routing weights + scaled-H transpose; returns gt [ER, P] bf16
            def make_gt(t, h):
                w1s = small.tile([P, 1], f32, tag="w1s", name=f"w1s_{t}")
                nc.scalar.activation(w1s[:], h[:, ER:ER + 1],
                                     mybir.ActivationFunctionType.Sigmoid,
                                     bias=brd128[:, 0:1], scale=1.0)
                w0s = small.tile([P, 1], f32, tag="w0s", name=f"w0s_{t}")
                nc.scalar.activation(w0s[:], h[:, ER:ER + 1],
                                     mybir.ActivationFunctionType.Sigmoid,
                                     bias=brd128[:, 1:2], scale=-1.0)
                g = small.tile([P, ER], f32, tag="g", name=f"g_{t}")
                nc.vector.tensor_scalar_mul(g[:, 0:8], h[:, 0:8], w0s[:])
                nc.vector.tensor_scalar_mul(g[:, 8:16], h[:, 8:16], w1s[:])
                gst = psT.tile([ER, P], f32, tag="gst", name=f"gst_{t}")
                nc.tensor.transpose(gst[:], g[:], ident[:])
                gt = small.tile([ER, P], bf16, tag="gt", name=f"gt_{t}")
                nc.vector.tensor_copy(gt[:], gst[:])
                return gt

            # h pre-block: open the h accumulation with k-slabs 14,15 so the
            # in-loop h stops at k=13, two matmul groups before the loop ends.
            def h_preblock(t, h):
                nc.tensor.matmul(h[:], lhs(t, KT - 2),
                                 aat_b[:, (KT - 2) * HN:(KT - 1) * HN],
                                 start=True, stop=False)
                nc.tensor.matmul(h[:], lhs(t, KT - 1),
                                 aat_b[:, (KT - 1) * HN:KT * HN],
                                 start=False, stop=False)

            # drain acc j of tile t as a pure copy (bias added on host);
            # even j on DVE, odd j on ACT, so the two engines split the work.
            def drain(t, acc, j):
                outt = out_tiles[t]
                dst = outt[:, j * 512:(j + 1) * 512]
                if j % 2 == 0:
                    nc.vector.tensor_copy(dst, acc[:])
                else:
                    nc.scalar.copy(dst, acc[:])

            def store(t, half):
                nc.gpsimd.dma_start(
                    out_d[t * P:(t + 1) * P, half * HO:(half + 1) * HO],
                    out_tiles[t][:, half * HO:(half + 1) * HO])

            out_tiles = [None] * NT

            # =========== startup: tiles 0,1 as a pair, half-O per pass =====
            hps = [psH.tile([P, HN], f32, tag="h", name=f"h_{t}")
                   for t in range(2)]
            gts = [None, None]
            for t in range(2):
                out_tiles[t] = obuf.tile([P, O], bf16, tag="obuf", name=f"out_{t}")

            STAG = 4  # phase-A stagger: t1 trails t0 by 4 k-groups

            def pair_mm(hh, accs, t, k):
                if hh == 0 and k < KT - 2:
                    # h opens at k=0 (the k=14,15 contributions are added
                    # mid-stream once the tail x chunks land) and stops at
                    # k=13 so the make_gt chain overlaps matmul groups
                    nc.tensor.matmul(hps[t][:], lhs(t, k),
                                     aat_b[:, k * HN:(k + 1) * HN],
                                     start=(k == 0), stop=(k == KT - 3))
                for j in range(2):
                    nc.tensor.matmul(
                        accs[t][j][:], lhs(t, k),
                        wt_b[:, k * O + hh * HO + j * 512:
                             k * O + hh * HO + (j + 1) * 512],
                        start=(k == 0), stop=False)
                if hh == 0 and k == 3:
                    nc.tensor.matmul(hps[t][:], lhs(t, KT - 2),
                                     aat_b[:, (KT - 2) * HN:(KT - 1) * HN],
                                     start=False, stop=False)
                    nc.tensor.matmul(hps[t][:], lhs(t, KT - 1),
                                     aat_b[:, (KT - 1) * HN:KT * HN],
                                     start=False, stop=False)
                if hh == 0 and k == KT - 2:
                    gts[t] = make_gt(t, hps[t])

            for hh in range(2):
                accs = [[psA.tile([P, 512], f32, tag="acc", name=f"acc_{t}_{hh}_{j}")
                         for j in range(2)] for t in range(2)]
                if hh == 0:
                    # t1's x tile lands ~2us after t0's on the serial ACT
                    # queue; staggering its strand lets the PE fill that
                    # window with t0's k=1..3 instead of stalling (and
                    # resetting the p-state clock ramp).
                    for k in range(KT + STAG):
                        if k < KT:
                            pair_mm(hh, accs, 0, k)
                        if k >= STAG:
                            pair_mm(hh, accs, 1, k - STAG)
                else:
                    for k in range(KT):
                        for t in range(2):
                            pair_mm(hh, accs, t, k)
                for t in range(2):
                    for j in range(2):
                        nc.tensor.matmul(accs[t][j][:], gts[t][:],
                                         bt_b[:, hh * HO + j * 512:
                                              hh * HO + (j + 1) * 512],
                                         start=False, stop=True)
                for t in range(2):
                    for j in range(2):
                        drain(t, accs[t][j], 2 * hh + j)
                for t in range(2):
                    store(t, hh)

            load_x(2)
            load_x(3)

            # =========== main loop: tiles 2..15, one tile at a time ========
            for t in range(2, NT):
                if t + 2 < NT:
                    load_x(t + 2)
                out_tiles[t] = obuf.tile([P, O], bf16, tag="obuf", name=f"out_{t}")
                accs = [psA.tile([P, 512], f32, tag="acc", name=f"acc_{t}_{j}")
                        for j in range(4)]
                h = psH.tile([P, HN], f32, tag="h", name=f"h_{t}")
                h_preblock(t, h)
                gt = None
                for k in range(KT):
                    if k < KT - 2:
                        nc.tensor.matmul(h[:], lhs(t, k),
                                         aat_b[:, k * HN:(k + 1) * HN],
                                         start=False, stop=(k == KT - 3))
                    for j in range(4):
                        nc.tensor.matmul(
                            accs[j][:], lhs(t, k),
                            wt_b[:, k * O + j * 512:k * O + (j + 1) * 512],
                            start=(k == 0), stop=False)
                    if k == KT - 2:
                        gt = make_gt(t, h)
                for j in range(4):
                    nc.tensor.matmul(accs[j][:], gt[:],
                                     bt_b[:, j * 512:(j + 1) * 512],
                                     start=False, stop=True)
                # drain order 0(DVE),1(ACT),2(DVE),3(ACT); store each half
                # as soon as both of its column chunks are in SBUF. The
                # last tile stores per-quarter to shorten the drain tail.
                if t == NT - 1:
                    # quarter stores on two idle queues -> parallel transfers
                    for j, q in zip(range(4),
                                    (nc.gpsimd, nc.sync, nc.gpsimd, nc.sync)):
                        drain(t, accs[j], j)
                        q.dma_start(
                            out_d[t * P:(t + 1) * P, j * 512:(j + 1) * 512],
                            out_tiles[t][:, j * 512:(j + 1) * 512])
                else:
                    drain(t, accs[0], 0)
                    drain(t, accs[1], 1)
                    store(t, 0)
                    drain(t, accs[2], 2)
                    drain(t, accs[3], 3)
                    store(t, 1)

    nc.compile()
    return nc


def _prep_host(x, W_base, b_base, A, B, W_router, b_router):
    """Host-side layout prep + sharding. Returns per-core input maps."""
    import ml_dtypes
    bf16 = ml_dtypes.bfloat16

    x_flat = np.ascontiguousarray(x, dtype=np.float32).reshape(-1, D)
    # xt[t*P + p, k*P + j] = x[t*P + j, k*P + p], per core
    NTOT = x_flat.shape[0] // P
    xt_all = np.ascontiguousarray(
        x_flat.reshape(NTOT, P, KT, P).transpose(0, 3, 2, 1)
    ).reshape(NTOT * P, D).astype(bf16)

    wt = np.asarray(W_base, dtype=np.float32).T                      # [D, O]
    wt_p = np.ascontiguousarray(
        wt.reshape(KT, P, O).transpose(1, 0, 2).reshape(P, KT * O)
    ).astype(bf16)

    a_cat = np.asarray(A, dtype=np.float32).reshape(ER, D)           # [16, D]
    aat = np.zeros((D, HN), dtype=np.float32)
    aat[:, :ER] = a_cat.T
    wr = np.asarray(W_router, dtype=np.float32)
    aat[:, ER] = wr[:, 1] - wr[:, 0]
    aat_p = np.ascontiguousarray(
        aat.reshape(KT, P, HN).transpose(1, 0, 2).reshape(P, KT * HN)
    ).astype(bf16)

    b_cat = np.concatenate([np.asarray(B, dtype=np.float32)[0],
                            np.asarray(B, dtype=np.float32)[1]], axis=1)  # [O, 16]
    bt = np.ascontiguousarray(b_cat.T * SCALE).astype(bf16)          # [16, O]
    dlb = np.float32(b_router[1]) - np.float32(b_router[0])
    brd = np.array([[dlb, -dlb]], dtype=np.float32)

    in_maps = []
    for c in range(NCORES):
        in_maps.append({
            "xt": xt_all[c * TOK:(c + 1) * TOK],
            "wt": wt_p,
            "aat": aat_p,
            "bt": bt,
            "brd": brd,
        })
    return in_maps


def kernel(x, W_base, b_base, A, B, W_router, b_router):
    from concourse import bass_utils

    # NOTE: walrus --enable-ldw-opt=true rejects bf16 LDWEIGHTS
    # ("InstLdweights is not compatible with LDW optimization"); measured
    # on HW the un-deduped loads fully hide under the matmul streams.
    if "nc" not in _CACHE:
        _CACHE["nc"] = _build()
    nc = _CACHE["nc"]

    in_maps = _prep_host(x, W_base, b_base, A, B, W_router, b_router)
    res = None
    for attempt in range(3):
        try:
            res = bass_utils.run_bass_kernel_spmd(
                nc, in_maps, core_ids=list(range(NCORES)))
            break
        except Exception:
            # rare transient NRT_EXEC_UNIT_UNRECOVERABLE observed once;
            # the same NEFF runs fine on retry
            if attempt == 2:
                raise
    out = np.concatenate([res.results[c]["out"] for c in range(NCORES)], axis=0)
    out = out.astype(np.float32) + np.asarray(b_base, dtype=np.float32)
    return out.reshape(np.asarray(x).shape[0], -1, O)
